# revision 30
# baseline (speedup 1.0000x reference)
"""Causal single-head attention (B=4, T=2048, E=1024, D=128) on 8 TRN2 cores.

Sharding: core c = (b, h) with b = c // 2, h = c % 2. Each core handles batch b
and 4 query "slots" i=0..3: queries [512*i + 256*h, +256), keys [0, 512*(i+1))
(rectangularized causal; exact causality via data-driven multiplicative masks).
All cores run ONE identical bass program; per-core differences are expressed
purely via host-prepared DRAM input data.

Per core (all matmuls float32r):
  1. K/V projections over all 2048 tokens from host-pre-transposed, pre-packed
     xT tiles (contraction dim e on partitions, fully contiguous DMA).
  2. RoPE: raw k evicted to SBUF, partition-pair-swapped via 2 stride-2
     SBUF->SBUF DMAs, combined on DVE: k' = k*cosT + kswap*sinT.
  3. V^T -> V natural via PE transposes.
  4. Per slot: S^T chunk = k'^T_chunk.T @ q'^T -> exp on ACT -> mask mul ->
     ones-matmul denominator + AV matmul (separate PSUM banks) ->
     reciprocal+normalize on DVE -> PE transpose -> out.
"""

import sys

for _p in ("/opt/trn_rl_repo",):
    if _p not in sys.path:
        sys.path.insert(0, _p)

import numpy as np

# run_bass_kernel_spmd imports antenv.axon_hooks only on the trace path; if the
# environment sets BASS_TRACE but lacks the module, provide a no-op shim.
try:
    import antenv.axon_hooks  # noqa: F401
except Exception:
    import types as _types

    _m = _types.ModuleType("antenv.axon_hooks")
    _m.set_axon_ntff_profile_hook = lambda h: None
    _m.get_axon_ntff_profile_hook = lambda: None
    sys.modules.setdefault("antenv.axon_hooks", _m)

import concourse.bacc as bacc
import concourse.mybir as mybir
import concourse.tile as tile
from concourse.bass_utils import run_bass_kernel_spmd
from concourse.masks import make_identity
import concourse.bass_isa as bass_isa

F32 = mybir.dt.float32
F32R = mybir.dt.float32r

B, T, E, D = 4, 2048, 1024, 128
THETA = 10000.0
SCALE = 1.0 / np.sqrt(np.float32(D))
N_CORES = 8
N_SLOTS = 4
SLOT_Q = 256
KV_CH = T // 128
N_TC = T // 512
N_EC = E // 128


def _build_nc():
    nc = bacc.Bacc(None, target_bir_lowering=False, debug=False)

    # pre-packed inputs: [partition, ...] layouts, fully contiguous per row
    wk = nc.dram_tensor("wk", [128, N_EC, D], F32R, kind="ExternalInput")
    wv = nc.dram_tensor("wv", [128, N_EC, D], F32R, kind="ExternalInput")
    wq = nc.dram_tensor("wq", [128, N_EC, D], F32R, kind="ExternalInput")
    xt_d = nc.dram_tensor("xt", [128, N_TC, N_EC, 512], F32R, kind="ExternalInput")
    xq_d = nc.dram_tensor("xq", [128, N_SLOTS, N_EC, SLOT_Q], F32R, kind="ExternalInput")
    ctabK = nc.dram_tensor("ctabK", [D, T], F32, kind="ExternalInput")
    stabK = nc.dram_tensor("stabK", [D, T], F32, kind="ExternalInput")
    ctabQ = nc.dram_tensor("ctabQ", [D, N_SLOTS * SLOT_Q], F32, kind="ExternalInput")
    stabQ = nc.dram_tensor("stabQ", [D, N_SLOTS * SLOT_Q], F32, kind="ExternalInput")
    masks = nc.dram_tensor("masks", [128, 4, SLOT_Q], F32, kind="ExternalInput")
    ones_d = nc.dram_tensor("ones", [128, 128], F32R, kind="ExternalInput")
    out_d = nc.dram_tensor("out", [D, N_SLOTS * SLOT_Q], F32, kind="ExternalOutput")

    with tile.TileContext(nc) as tc:
        with (
            tc.tile_pool(name="const", bufs=1) as const,
            tc.tile_pool(name="persist", bufs=1) as persist,
            tc.tile_pool(name="work", bufs=2) as work,
            tc.tile_pool(name="pp", bufs=1, space="PSUM") as pp,
            tc.tile_pool(name="ps", bufs=4, space="PSUM") as ps,
            tc.tile_pool(name="pa", bufs=1, space="PSUM") as pa,
        ):
            # sync queue: wk wv xt0(split) xt1..3 (kv critical path)
            # scalar queue: wq ones xq0 k-tables q-tables masks xq1..3
            w_sb = {}
            for name, dram, eng in (("k", wk, nc.gpsimd), ("v", wv, nc.gpsimd), ("q", wq, nc.scalar)):
                t = const.tile([128, N_EC, D], F32R, tag=f"w_{name}")
                eng.dma_start(out=t[:, 0:4], in_=dram[:, 0:4])
                eng.dma_start(out=t[:, 4:8], in_=dram[:, 4:8])
                w_sb[name] = t

            xt = persist.tile([128, N_TC, N_EC, 512], F32R)
            for ec in range(N_EC):
                nc.sync.dma_start(out=xt[:, 0, ec], in_=xt_d[:, 0, ec])
            for tci in range(1, N_TC):
                nc.sync.dma_start(out=xt[:, tci], in_=xt_d[:, tci])

            xtq = persist.tile([128, N_SLOTS, N_EC, SLOT_Q], F32R)
            nc.scalar.dma_start(out=xtq[:, 0], in_=xq_d[:, 0])
            mask_sb = const.tile([128, 4, SLOT_Q], F32)
            nc.scalar.dma_start(out=mask_sb, in_=masks[:])
            for si in range(1, N_SLOTS):
                nc.scalar.dma_start(out=xtq[:, si], in_=xq_d[:, si])
            ones = const.tile([128, 128], F32R)
            nc.scalar.dma_start(out=ones, in_=ones_d[:])
            ctabK_sb = const.tile([D, T], F32)
            nc.scalar.dma_start(out=ctabK_sb, in_=ctabK[:])
            stabK_sb = const.tile([D, T], F32)
            nc.scalar.dma_start(out=stabK_sb, in_=stabK[:])
            ctabQ_sb = const.tile([D, N_SLOTS * SLOT_Q], F32)
            nc.scalar.dma_start(out=ctabQ_sb, in_=ctabQ[:])
            stabQ_sb = const.tile([D, N_SLOTS * SLOT_Q], F32)
            nc.scalar.dma_start(out=stabQ_sb, in_=stabQ[:])
            ident = const.tile([128, 128], F32)
            make_identity(nc, ident)

            kT_sb = persist.tile([D, T], F32R)
            qT_sb = persist.tile([D, N_SLOTS * SLOT_Q], F32R)
            v_nat = persist.tile([128, KV_CH, D], F32R)

            def rope(psum, width, ctab_ap, stab_ap, out_ap):
                raw = work.tile([128, width], F32, tag="raw")
                nc.vector.tensor_copy(raw, psum)
                sw = work.tile([128, width], F32, tag="sw")
                s2 = raw.rearrange("(a b) f -> a b f", b=2)
                d2 = sw.rearrange("(a b) f -> a b f", b=2)
                nc.gpsimd.dma_start(out=d2[:, 0, :], in_=s2[:, 1, :])
                nc.gpsimd.dma_start(out=d2[:, 1, :], in_=s2[:, 0, :])
                t1 = work.tile([128, width], F32, tag="ropeA")
                nc.vector.tensor_mul(t1, psum, ctab_ap)
                t2 = work.tile([128, width], F32, tag="ropeB")
                nc.vector.tensor_mul(t2, sw, stab_ap)
                nc.vector.tensor_add(out_ap, t1, t2)

            def kv_proj(tci):
                cs = slice(tci * 512, (tci + 1) * 512)
                psk = pp.tile([128, 512], F32, tag="psk")
                psv = pp.tile([128, 512], F32, tag="psv")
                for ec in range(N_EC):
                    st, sp = ec == 0, ec == N_EC - 1
                    nc.tensor.matmul(psk, w_sb["k"][:, ec, :], xt[:, tci, ec, :], start=st, stop=sp)
                    nc.tensor.matmul(psv, w_sb["v"][:, ec, :], xt[:, tci, ec, :], start=st, stop=sp)
                rope(psk, 512, ctabK_sb[:, cs], stabK_sb[:, cs], kT_sb[:, cs])
                vt = work.tile([128, 512], F32, tag="vt")
                nc.vector.tensor_copy(vt, psv)
                for j in range(4):
                    pt = ps.tile([128, 128], F32, tag="s")
                    nc.tensor.transpose(pt, vt[:, j * 128:(j + 1) * 128], ident)
                    nc.scalar.copy(v_nat[:, tci * 4 + j, :], pt)

            def q_proj(si):
                qs = slice(si * SLOT_Q, (si + 1) * SLOT_Q)
                psq = pp.tile([128, SLOT_Q], F32, tag="psk")
                for ec in range(N_EC):
                    nc.tensor.matmul(psq, w_sb["q"][:, ec, :], xtq[:, si, ec, :],
                                     start=ec == 0, stop=ec == N_EC - 1)
                rope(psq, SLOT_Q, ctabQ_sb[:, qs], stabQ_sb[:, qs], qT_sb[:, qs])

            for i in range(N_TC):
                q_proj(i)
                kv_proj(i)

            # ---- Attention (narrow slots, deep S lookahead) ----
            for si in range(N_SLOTS):
                qs = slice(si * SLOT_Q, (si + 1) * SLOT_Q)
                n_ch = 4 * (si + 1)
                pacc_av = pa.tile([128, SLOT_Q], F32, tag="pacc_av")
                pacc_d = pa.tile([128, SLOT_Q], F32, tag="pacc_d")
                for c in range(n_ch):
                    pss = ps.tile([128, SLOT_Q], F32, tag="s")
                    nc.tensor.matmul(pss, kT_sb[:, c * 128:(c + 1) * 128], qT_sb[:, qs],
                                     start=True, stop=True)
                    pT = work.tile([128, SLOT_Q], F32R, tag="pT", bufs=4)
                    nc.scalar.activation(out=pT, in_=pss,
                                         func=mybir.ActivationFunctionType.Exp, scale=float(SCALE))
                    j = c - (n_ch - 4)
                    if j >= 0:
                        nc.vector.tensor_mul(pT, pT, mask_sb[:, j, :])
                    st, sp = c == 0, c == n_ch - 1
                    nc.tensor.matmul(pacc_d, ones, pT, start=st, stop=sp)
                    nc.tensor.matmul(pacc_av, v_nat[:, c, :], pT, start=st, stop=sp)
                recip = work.tile([128, SLOT_Q], F32, tag="recip")
                nc.vector.reciprocal(recip, pacc_d)
                oT = work.tile([128, SLOT_Q], F32, tag="oT")
                nc.vector.tensor_mul(oT, pacc_av, recip)
                # out stays d-major; host transposes during unshard
                nc.sync.dma_start(out=out_d[:, si * SLOT_Q:(si + 1) * SLOT_Q], in_=oT)
    nc.compile()
    return nc


_NC = None


def _get_nc():
    global _NC
    if _NC is None:
        _NC = _build_nc()
    return _NC


def _host_prep(embedding_word, w_Q, w_K, w_V):
    x = np.asarray(embedding_word, dtype=np.float32)
    w_Q = np.asarray(w_Q, dtype=np.float32)
    w_K = np.asarray(w_K, dtype=np.float32)
    w_V = np.asarray(w_V, dtype=np.float32)

    # packed weights: [p, ec, d] = W.T[ec*128+p, d]
    def pack_w(w):
        return np.ascontiguousarray(w.T.reshape(N_EC, 128, D).transpose(1, 0, 2))

    wq_p, wk_p, wv_p = pack_w(w_Q), pack_w(w_K), pack_w(w_V)

    # RoPE tables in [d, t] layout
    j = np.arange(D // 2, dtype=np.float64)
    freqs = 1.0 / THETA ** (2.0 * j / D)
    t = np.arange(T, dtype=np.float64)
    ang = np.outer(freqs, t)
    cos = np.cos(ang)
    sin = np.sin(ang)
    ctab = np.repeat(cos, 2, axis=0).astype(np.float32)
    stab = np.empty((D, T), dtype=np.float32)
    stab[0::2] = -sin
    stab[1::2] = sin

    qcols = {}
    for h in (0, 1):
        qcols[h] = np.concatenate([np.arange(512 * i + 256 * h, 512 * i + 256 * h + SLOT_Q)
                                   for i in range(N_SLOTS)])

    masks_h = {}
    for h in (0, 1):
        m = np.empty((4, 128, SLOT_Q), dtype=np.float32)
        for jj in range(4):
            xg, yg = np.meshgrid(np.arange(128), np.arange(SLOT_Q), indexing="ij")
            m[jj] = ((yg - xg) >= (128 * jj - 256 * h)).astype(np.float32)
        # pack to [p, j, y]
        masks_h[h] = np.ascontiguousarray(m.transpose(1, 0, 2))

    in_maps = []
    for c in range(N_CORES):
        b, h = c // 2, c % 2
        xT = x[b].T  # [E, T]
        # xt packed [p, tc, ec, t] = xT[ec*128+p, tc*512+t]
        xt_p = np.ascontiguousarray(
            xT.reshape(N_EC, 128, N_TC, 512).transpose(1, 2, 0, 3))
        xq = xT[:, qcols[h]]  # [E, 1024]
        xq_p = np.ascontiguousarray(
            xq.reshape(N_EC, 128, N_SLOTS, SLOT_Q).transpose(1, 2, 0, 3))
        in_maps.append({
            "xt": xt_p, "xq": xq_p,
            "wq": wq_p, "wk": wk_p, "wv": wv_p,
            "ctabK": ctab, "stabK": stab,
            "ctabQ": np.ascontiguousarray(ctab[:, qcols[h]]),
            "stabQ": np.ascontiguousarray(stab[:, qcols[h]]),
            "masks": masks_h[h],
            "ones": np.ones((128, 128), dtype=np.float32),
        })
    return in_maps


def _assemble(results):
    out = np.empty((B, T, D), dtype=np.float32)
    for c in range(N_CORES):
        b, h = c // 2, c % 2
        o = results[c]["out"]  # [D, 1024], d-major
        for i in range(N_SLOTS):
            out[b, 512 * i + 256 * h: 512 * i + 256 * h + SLOT_Q, :] = \
                o[:, i * SLOT_Q:(i + 1) * SLOT_Q].T
    return out


def run(inputs, trace=False, tmpdir=None):
    nc = _get_nc()
    in_maps = _host_prep(**inputs)
    res = run_bass_kernel_spmd(nc, in_maps, list(range(N_CORES)), trace=trace, tmpdir=tmpdir)
    return _assemble(res.results), res


def kernel(embedding_word, w_Q, w_K, w_V):
    out, _ = run(dict(embedding_word=embedding_word, w_Q=w_Q, w_K=w_K, w_V=w_V))
    return out


# revision 31
# speedup vs baseline: 1.0342x; 1.0342x over previous
"""Causal single-head attention (B=4, T=2048, E=1024, D=128) on 8 TRN2 cores.

Sharding: core c = (b, h) with b = c // 2, h = c % 2. Each core handles batch b
and 4 query "slots" i=0..3: queries [512*i + 256*h, +256), keys [0, 512*(i+1))
(rectangularized causal; exact causality via data-driven multiplicative masks).
All cores run ONE identical bass program; per-core differences are expressed
purely via host-prepared DRAM input data.

Per core (all matmuls float32r):
  1. K/V projections over all 2048 tokens from host-pre-transposed, pre-packed
     xT tiles (contraction dim e on partitions, fully contiguous DMA).
  2. RoPE: raw k evicted to SBUF, partition-pair-swapped via 2 stride-2
     SBUF->SBUF DMAs, combined on DVE: k' = k*cosT + kswap*sinT.
  3. V^T -> V natural via PE transposes.
  4. Per slot: S^T chunk = k'^T_chunk.T @ q'^T -> exp on ACT -> mask mul ->
     ones-matmul denominator + AV matmul (separate PSUM banks) ->
     reciprocal+normalize on DVE -> PE transpose -> out.
"""

import sys

for _p in ("/opt/trn_rl_repo",):
    if _p not in sys.path:
        sys.path.insert(0, _p)

import numpy as np

# run_bass_kernel_spmd imports antenv.axon_hooks only on the trace path; if the
# environment sets BASS_TRACE but lacks the module, provide a no-op shim.
try:
    import antenv.axon_hooks  # noqa: F401
except Exception:
    import types as _types

    _m = _types.ModuleType("antenv.axon_hooks")
    _m.set_axon_ntff_profile_hook = lambda h: None
    _m.get_axon_ntff_profile_hook = lambda: None
    sys.modules.setdefault("antenv.axon_hooks", _m)

import concourse.bacc as bacc
import concourse.mybir as mybir
import concourse.tile as tile
from concourse.bass_utils import run_bass_kernel_spmd
from concourse.masks import make_identity
import concourse.bass_isa as bass_isa

F32 = mybir.dt.float32
F32R = mybir.dt.float32r

B, T, E, D = 4, 2048, 1024, 128
THETA = 10000.0
SCALE = 1.0 / np.sqrt(np.float32(D))
N_CORES = 8
N_SLOTS = 4
SLOT_Q = 256
KV_CH = T // 128
N_TC = T // 512
N_EC = E // 128


def _build_nc():
    nc = bacc.Bacc(None, target_bir_lowering=False, debug=False)

    # pre-packed inputs: [partition, ...] layouts, fully contiguous per row
    wk = nc.dram_tensor("wk", [128, N_EC, D], F32R, kind="ExternalInput")
    wv = nc.dram_tensor("wv", [128, N_EC, D], F32R, kind="ExternalInput")
    wq = nc.dram_tensor("wq", [128, N_EC, D], F32R, kind="ExternalInput")
    xt_d = nc.dram_tensor("xt", [128, N_TC, N_EC, 512], F32R, kind="ExternalInput")
    xq_d = nc.dram_tensor("xq", [128, N_SLOTS, N_EC, SLOT_Q], F32R, kind="ExternalInput")
    ctabK = nc.dram_tensor("ctabK", [D, T], F32, kind="ExternalInput")
    stabK = nc.dram_tensor("stabK", [D, T], F32, kind="ExternalInput")
    ctabQ = nc.dram_tensor("ctabQ", [D, N_SLOTS * SLOT_Q], F32, kind="ExternalInput")
    stabQ = nc.dram_tensor("stabQ", [D, N_SLOTS * SLOT_Q], F32, kind="ExternalInput")
    masks = nc.dram_tensor("masks", [128, 4, SLOT_Q], F32, kind="ExternalInput")
    ones_d = nc.dram_tensor("ones", [128, 128], F32R, kind="ExternalInput")
    out_d = nc.dram_tensor("out", [D, N_SLOTS * SLOT_Q], F32, kind="ExternalOutput")

    with tile.TileContext(nc) as tc:
        with (
            tc.tile_pool(name="const", bufs=1) as const,
            tc.tile_pool(name="persist", bufs=1) as persist,
            tc.tile_pool(name="work", bufs=2) as work,
            tc.tile_pool(name="pp", bufs=1, space="PSUM") as pp,
            tc.tile_pool(name="ps", bufs=4, space="PSUM") as ps,
            tc.tile_pool(name="pa", bufs=1, space="PSUM") as pa,
        ):
            # sync queue: wk wv xt0(split) xt1..3 (kv critical path)
            # scalar queue: wq ones xq0 k-tables q-tables masks xq1..3
            w_sb = {}
            for name, dram, eng in (("k", wk, nc.gpsimd), ("v", wv, nc.gpsimd), ("q", wq, nc.scalar)):
                t = const.tile([128, N_EC, D], F32R, tag=f"w_{name}")
                eng.dma_start(out=t[:, 0:4], in_=dram[:, 0:4])
                eng.dma_start(out=t[:, 4:8], in_=dram[:, 4:8])
                w_sb[name] = t

            xt = persist.tile([128, N_TC, N_EC, 512], F32R)
            for ec in range(N_EC):
                nc.sync.dma_start(out=xt[:, 0, ec], in_=xt_d[:, 0, ec])
            for tci in range(1, N_TC):
                nc.sync.dma_start(out=xt[:, tci], in_=xt_d[:, tci])

            xtq = persist.tile([128, N_SLOTS, N_EC, SLOT_Q], F32R)
            nc.scalar.dma_start(out=xtq[:, 0], in_=xq_d[:, 0])
            mask_sb = const.tile([128, 4, SLOT_Q], F32)
            nc.scalar.dma_start(out=mask_sb, in_=masks[:])
            for si in range(1, N_SLOTS):
                nc.scalar.dma_start(out=xtq[:, si], in_=xq_d[:, si])
            ones = const.tile([128, 128], F32R)
            nc.scalar.dma_start(out=ones, in_=ones_d[:])
            ctabK_sb = const.tile([D, T], F32)
            nc.scalar.dma_start(out=ctabK_sb, in_=ctabK[:])
            stabK_sb = const.tile([D, T], F32)
            nc.scalar.dma_start(out=stabK_sb, in_=stabK[:])
            ctabQ_sb = const.tile([D, N_SLOTS * SLOT_Q], F32)
            nc.scalar.dma_start(out=ctabQ_sb, in_=ctabQ[:])
            stabQ_sb = const.tile([D, N_SLOTS * SLOT_Q], F32)
            nc.scalar.dma_start(out=stabQ_sb, in_=stabQ[:])
            ident = const.tile([128, 128], F32)
            make_identity(nc, ident)

            kT_sb = persist.tile([D, T], F32R)
            qT_sb = persist.tile([D, N_SLOTS * SLOT_Q], F32R)
            v_nat = persist.tile([128, KV_CH, D], F32R)

            def rope(psum, width, ctab_ap, stab_ap, out_ap):
                raw = work.tile([128, width], F32, tag="raw")
                nc.vector.tensor_copy(raw, psum)
                sw = work.tile([128, width], F32, tag="sw")
                s2 = raw.rearrange("(a b) f -> a b f", b=2)
                d2 = sw.rearrange("(a b) f -> a b f", b=2)
                nc.gpsimd.dma_start(out=d2[:, 0, :], in_=s2[:, 1, :])
                nc.gpsimd.dma_start(out=d2[:, 1, :], in_=s2[:, 0, :])
                t1 = work.tile([128, width], F32, tag="ropeA")
                nc.vector.tensor_mul(t1, psum, ctab_ap)
                t2 = work.tile([128, width], F32, tag="ropeB")
                nc.vector.tensor_mul(t2, sw, stab_ap)
                nc.vector.tensor_add(out_ap, t1, t2)

            def kv_proj(tci):
                cs = slice(tci * 512, (tci + 1) * 512)
                psk = pp.tile([128, 512], F32, tag="psk")
                psv = pp.tile([128, 512], F32, tag="psv")
                for ec in range(N_EC):
                    st, sp = ec == 0, ec == N_EC - 1
                    nc.tensor.matmul(psk, w_sb["k"][:, ec, :], xt[:, tci, ec, :], start=st, stop=sp)
                    nc.tensor.matmul(psv, w_sb["v"][:, ec, :], xt[:, tci, ec, :], start=st, stop=sp)
                rope(psk, 512, ctabK_sb[:, cs], stabK_sb[:, cs], kT_sb[:, cs])
                vt = work.tile([128, 512], F32, tag="vt")
                nc.vector.tensor_copy(vt, psv)
                for j in range(4):
                    pt = ps.tile([128, 128], F32, tag="s")
                    nc.tensor.transpose(pt, vt[:, j * 128:(j + 1) * 128], ident)
                    nc.scalar.copy(v_nat[:, tci * 4 + j, :], pt)

            def q_proj(si):
                qs = slice(si * SLOT_Q, (si + 1) * SLOT_Q)
                psq = pp.tile([128, SLOT_Q], F32, tag="psk")
                for ec in range(N_EC):
                    nc.tensor.matmul(psq, w_sb["q"][:, ec, :], xtq[:, si, ec, :],
                                     start=ec == 0, stop=ec == N_EC - 1)
                rope(psq, SLOT_Q, ctabQ_sb[:, qs], stabQ_sb[:, qs], qT_sb[:, qs])

            for i in range(N_TC):
                kv_proj(i)
                q_proj(i)

            # ---- Attention (narrow slots, deep S lookahead) ----
            for si in range(N_SLOTS):
                qs = slice(si * SLOT_Q, (si + 1) * SLOT_Q)
                n_ch = 4 * (si + 1)
                pacc_av = pa.tile([128, SLOT_Q], F32, tag="pacc_av")
                pacc_d = pa.tile([128, SLOT_Q], F32, tag="pacc_d")
                for c in range(n_ch):
                    pss = ps.tile([128, SLOT_Q], F32, tag="s")
                    nc.tensor.matmul(pss, kT_sb[:, c * 128:(c + 1) * 128], qT_sb[:, qs],
                                     start=True, stop=True)
                    pT = work.tile([128, SLOT_Q], F32R, tag="pT", bufs=4)
                    nc.scalar.activation(out=pT, in_=pss,
                                         func=mybir.ActivationFunctionType.Exp, scale=float(SCALE))
                    j = c - (n_ch - 4)
                    if j >= 0:
                        nc.vector.tensor_mul(pT, pT, mask_sb[:, j, :])
                    st, sp = c == 0, c == n_ch - 1
                    nc.tensor.matmul(pacc_d, ones, pT, start=st, stop=sp)
                    nc.tensor.matmul(pacc_av, v_nat[:, c, :], pT, start=st, stop=sp)
                recip = work.tile([128, SLOT_Q], F32, tag="recip")
                nc.vector.reciprocal(recip, pacc_d)
                oT = work.tile([128, SLOT_Q], F32, tag="oT")
                nc.vector.tensor_mul(oT, pacc_av, recip)
                # out stays d-major; host transposes during unshard
                nc.sync.dma_start(out=out_d[:, si * SLOT_Q:(si + 1) * SLOT_Q], in_=oT)
    nc.compile()
    return nc


_NC = None


def _get_nc():
    global _NC
    if _NC is None:
        _NC = _build_nc()
    return _NC


def _host_prep(embedding_word, w_Q, w_K, w_V):
    x = np.asarray(embedding_word, dtype=np.float32)
    w_Q = np.asarray(w_Q, dtype=np.float32)
    w_K = np.asarray(w_K, dtype=np.float32)
    w_V = np.asarray(w_V, dtype=np.float32)

    # packed weights: [p, ec, d] = W.T[ec*128+p, d]
    def pack_w(w):
        return np.ascontiguousarray(w.T.reshape(N_EC, 128, D).transpose(1, 0, 2))

    wq_p, wk_p, wv_p = pack_w(w_Q), pack_w(w_K), pack_w(w_V)

    # RoPE tables in [d, t] layout
    j = np.arange(D // 2, dtype=np.float64)
    freqs = 1.0 / THETA ** (2.0 * j / D)
    t = np.arange(T, dtype=np.float64)
    ang = np.outer(freqs, t)
    cos = np.cos(ang)
    sin = np.sin(ang)
    ctab = np.repeat(cos, 2, axis=0).astype(np.float32)
    stab = np.empty((D, T), dtype=np.float32)
    stab[0::2] = -sin
    stab[1::2] = sin

    qcols = {}
    for h in (0, 1):
        qcols[h] = np.concatenate([np.arange(512 * i + 256 * h, 512 * i + 256 * h + SLOT_Q)
                                   for i in range(N_SLOTS)])

    masks_h = {}
    for h in (0, 1):
        m = np.empty((4, 128, SLOT_Q), dtype=np.float32)
        for jj in range(4):
            xg, yg = np.meshgrid(np.arange(128), np.arange(SLOT_Q), indexing="ij")
            m[jj] = ((yg - xg) >= (128 * jj - 256 * h)).astype(np.float32)
        # pack to [p, j, y]
        masks_h[h] = np.ascontiguousarray(m.transpose(1, 0, 2))

    in_maps = []
    for c in range(N_CORES):
        b, h = c // 2, c % 2
        xT = x[b].T  # [E, T]
        # xt packed [p, tc, ec, t] = xT[ec*128+p, tc*512+t]
        xt_p = np.ascontiguousarray(
            xT.reshape(N_EC, 128, N_TC, 512).transpose(1, 2, 0, 3))
        xq = xT[:, qcols[h]]  # [E, 1024]
        xq_p = np.ascontiguousarray(
            xq.reshape(N_EC, 128, N_SLOTS, SLOT_Q).transpose(1, 2, 0, 3))
        in_maps.append({
            "xt": xt_p, "xq": xq_p,
            "wq": wq_p, "wk": wk_p, "wv": wv_p,
            "ctabK": ctab, "stabK": stab,
            "ctabQ": np.ascontiguousarray(ctab[:, qcols[h]]),
            "stabQ": np.ascontiguousarray(stab[:, qcols[h]]),
            "masks": masks_h[h],
            "ones": np.ones((128, 128), dtype=np.float32),
        })
    return in_maps


def _assemble(results):
    out = np.empty((B, T, D), dtype=np.float32)
    for c in range(N_CORES):
        b, h = c // 2, c % 2
        o = results[c]["out"]  # [D, 1024], d-major
        for i in range(N_SLOTS):
            out[b, 512 * i + 256 * h: 512 * i + 256 * h + SLOT_Q, :] = \
                o[:, i * SLOT_Q:(i + 1) * SLOT_Q].T
    return out


def run(inputs, trace=False, tmpdir=None):
    nc = _get_nc()
    in_maps = _host_prep(**inputs)
    res = run_bass_kernel_spmd(nc, in_maps, list(range(N_CORES)), trace=trace, tmpdir=tmpdir)
    return _assemble(res.results), res


def kernel(embedding_word, w_Q, w_K, w_V):
    out, _ = run(dict(embedding_word=embedding_word, w_Q=w_Q, w_K=w_K, w_V=w_V))
    return out
